# revision 3
# baseline (speedup 1.0000x reference)
"""GRU decoder with categorical sampling on 8 TRN2 NeuronCores.

Sharding: fc_w is vocab-sharded (4000 rows/core) and SBUF-resident; GRU
weights are hidden-sharded (128 h rows/core). Per decode step: each core
computes its gate slice, the h' shards are AllGathered, each core computes
its logits shard against the resident fc_w shard, adds host-precomputed
gumbel noise (bit-exact match of jax.random.categorical's gumbel), takes a
per-shard argmax, AllGathers the 8 (value, index) candidates, reduces to the
global argmax, gathers the next input embedding with an indirect DMA, and
transposes it for the next step's GRU matmuls.
"""

import functools

import numpy as np

T, B, D, H, V = 50, 64, 512, 1024, 32000
NCORES = 8
VS = V // NCORES      # 4000 vocab rows per core
NB = 8                # logits bank count per step
BN = VS // NB         # 500 columns per bank (one PSUM bank)
KC_H = H // 128       # 8 contraction chunks over H
KC_D = D // 128       # 4 contraction chunks over D

COLPACK = False       # pack two K-chunks via tile_position (cols 0-63 / 64-127)


def _build_bass():
    import concourse.bass as bass
    import concourse.mybir as mybir
    import concourse.tile as tile
    from concourse import bacc
    from concourse.masks import make_identity

    f32 = mybir.dt.float32
    u32 = mybir.dt.uint32
    i32 = mybir.dt.int32

    nc = bacc.Bacc(num_devices=NCORES)
    RG = [list(range(NCORES))]

    # ---- kernel I/O ----
    fcT_d = nc.declare_dram_parameter("fcT", [128, KC_H * VS], f32, isOutput=False)
    wih_d = nc.declare_dram_parameter("wihT", [128, 3 * KC_D * 128], f32, isOutput=False)
    whh_d = nc.declare_dram_parameter("whhT", [128, 3 * KC_H * 128], f32, isOutput=False)
    bias_d = nc.declare_dram_parameter("bias", [128, 4], f32, isOutput=False)
    h0T_d = nc.declare_dram_parameter("h0T", [128, KC_H * B], f32, isOutput=False)
    h0s_d = nc.declare_dram_parameter("h0s", [128, B], f32, isOutput=False)
    x0T_d = nc.declare_dram_parameter("x0T", [128, KC_D * B], f32, isOutput=False)
    gum_d = nc.declare_dram_parameter("gum", [T, B, VS], f32, isOutput=False)
    emb_d = nc.declare_dram_parameter("emb", [V, D], f32, isOutput=False)

    logits_o = nc.declare_dram_parameter("logits_o", [T, B, VS], f32, isOutput=True)
    idx_o = nc.declare_dram_parameter("idx_o", [T, B], i32, isOutput=True)

    # ---- per-step collective buffers (static, no reuse -> no false deps) ----
    hb = [nc.dram_tensor(f"hb{t}", [128, B], f32) for t in range(T)]
    hg = [nc.dram_tensor(f"hg{t}", [H, B], f32, addr_space="Shared") for t in range(T)]
    cb = [nc.dram_tensor(f"cb{t}", [B, 2], f32) for t in range(T)]
    cg = [nc.dram_tensor(f"cg{t}", [B * NCORES, 2], f32, addr_space="Shared") for t in range(T)]

    sig = mybir.ActivationFunctionType.Sigmoid
    tanh = mybir.ActivationFunctionType.Tanh

    with tile.TileContext(nc) as tc:
        with (
            tc.tile_pool(name="const", bufs=1) as cpool,
            tc.tile_pool(name="state", bufs=2) as spool,
            tc.tile_pool(name="work", bufs=2) as wpool,
            tc.tile_pool(name="lg", bufs=1) as lgpool,
            tc.tile_pool(name="pslog", bufs=2, space="PSUM") as pslog,
            tc.tile_pool(name="psgh", bufs=2, space="PSUM") as psgh,
            tc.tile_pool(name="psgi", bufs=2, space="PSUM") as psgi,
            tc.tile_pool(name="psx", bufs=2, space="PSUM") as psx,
        ):
            # ---- load constants ----
            fcT = cpool.tile([128, KC_H, VS], f32, tag="fcT")
            for kc in range(KC_H):
                nc.gpsimd.dma_start(
                    out=fcT[:, kc, :], in_=fcT_d.ap()[:, kc * VS:(kc + 1) * VS]
                )
            wih = cpool.tile([128, 3, KC_D, 128], f32, tag="wih")
            nc.gpsimd.dma_start(out=wih[:], in_=wih_d.ap())
            whh = cpool.tile([128, 3, KC_H, 128], f32, tag="whh")
            nc.gpsimd.dma_start(out=whh[:], in_=whh_d.ap())
            bias = cpool.tile([128, 4], f32, tag="bias")
            nc.gpsimd.dma_start(out=bias[:], in_=bias_d.ap())
            ident = cpool.tile([128, 128], f32, tag="ident")
            make_identity(nc, ident[:])
            iota8 = cpool.tile([B, 8], i32, tag="iota8")
            nc.gpsimd.iota(out=iota8[:], pattern=[[1, 8]], base=0, channel_multiplier=0)
            iota8f = cpool.tile([B, 8], f32, tag="iota8f")
            nc.vector.tensor_copy(out=iota8f[:], in_=iota8[:])

            hT = spool.tile([128, KC_H, B], f32, tag="hT")
            nc.gpsimd.dma_start(out=hT[:], in_=h0T_d.ap())
            hsh = spool.tile([128, B], f32, tag="hsh")
            nc.gpsimd.dma_start(out=hsh[:], in_=h0s_d.ap())
            xT = spool.tile([128, KC_D, B], f32, tag="xT")
            nc.gpsimd.dma_start(out=xT[:], in_=x0T_d.ap())

            for t in range(T):
                # ---- GRU: h(t+1) = GRUCell(x_t, h_t) ----
                ps_gh = psgh.tile([128, 3, B], f32, tag="psgh")
                for g in range(3):
                    for kc in range(KC_H):
                        nc.tensor.matmul(
                            out=ps_gh[:, g, :],
                            lhsT=whh[:, g, kc, :],
                            rhs=hT[:, kc, :],
                            start=(kc == 0),
                            stop=(kc == KC_H - 1),
                        )
                ps_gi = psgi.tile([128, 3, B], f32, tag="psgi")
                for g in range(3):
                    for kc in range(KC_D):
                        nc.tensor.matmul(
                            out=ps_gi[:, g, :],
                            lhsT=wih[:, g, kc, :],
                            rhs=xT[:, kc, :],
                            start=(kc == 0),
                            stop=(kc == KC_D - 1),
                        )

                gt = wpool.tile([128, 6, B], f32, tag="gates")
                ghs = wpool.tile([128, 3, B], f32, tag="ghs")
                nc.scalar.copy(out=ghs[:], in_=ps_gh[:])
                # r = sigmoid(gi_r + gh_r + (b_ih_r + b_hh_r))
                nc.vector.tensor_tensor(
                    out=gt[:, 0, :], in0=ps_gi[:, 0, :], in1=ghs[:, 0, :],
                    op=mybir.AluOpType.add,
                )
                nc.scalar.activation(out=gt[:, 1, :], in_=gt[:, 0, :], func=sig,
                                     bias=bias[:, 0:1])
                # z = sigmoid(gi_z + gh_z + (b_ih_z + b_hh_z))
                nc.vector.tensor_tensor(
                    out=gt[:, 2, :], in0=ps_gi[:, 1, :], in1=ghs[:, 1, :],
                    op=mybir.AluOpType.add,
                )
                nc.scalar.activation(out=gt[:, 3, :], in_=gt[:, 2, :], func=sig,
                                     bias=bias[:, 1:2])
                # n = tanh(gi_n + b_ih_n + r * (gh_n + b_hh_n))
                nc.vector.tensor_scalar_add(out=gt[:, 4, :], in0=ghs[:, 2, :],
                                            scalar1=bias[:, 3:4])
                nc.vector.tensor_tensor(
                    out=gt[:, 4, :], in0=gt[:, 1, :], in1=gt[:, 4, :],
                    op=mybir.AluOpType.mult,
                )
                nc.vector.tensor_tensor(
                    out=gt[:, 4, :], in0=ps_gi[:, 2, :], in1=gt[:, 4, :],
                    op=mybir.AluOpType.add,
                )
                nc.scalar.activation(out=gt[:, 5, :], in_=gt[:, 4, :], func=tanh,
                                     bias=bias[:, 2:3])
                # h' = (1 - z) * n + z * h
                hsh_new = spool.tile([128, B], f32, tag="hsh")
                nc.vector.tensor_scalar(
                    out=gt[:, 0, :], in0=gt[:, 3, :], scalar1=-1.0, scalar2=1.0,
                    op0=mybir.AluOpType.mult, op1=mybir.AluOpType.add,
                )
                nc.vector.tensor_tensor(
                    out=gt[:, 0, :], in0=gt[:, 0, :], in1=gt[:, 5, :],
                    op=mybir.AluOpType.mult,
                )
                nc.vector.tensor_tensor(
                    out=gt[:, 2, :], in0=gt[:, 3, :], in1=hsh[:],
                    op=mybir.AluOpType.mult,
                )
                nc.vector.tensor_tensor(
                    out=hsh_new[:], in0=gt[:, 0, :], in1=gt[:, 2, :],
                    op=mybir.AluOpType.add,
                )
                hsh = hsh_new

                # ---- AllGather h' shards -> full h(t+1), transposed layout ----
                nc.gpsimd.dma_start(out=hb[t].ap(), in_=hsh[:])
                nc.gpsimd.collective_compute(
                    "AllGather", mybir.AluOpType.bypass, replica_groups=RG,
                    ins=[hb[t].ap()], outs=[hg[t].ap()],
                )
                hT = spool.tile([128, KC_H, B], f32, tag="hT")
                nc.gpsimd.dma_start(
                    out=hT[:], in_=hg[t].ap().rearrange("(c p) b -> p c b", p=128)
                )

                # ---- logits_t = h(t+1) @ fc_w_shard.T  (+ gumbel, sampled) ----
                logits_sb = lgpool.tile([B, NB, BN], f32, tag="logits")
                gum = lgpool.tile([B, NB, BN], f32, tag="gum")
                for n in range(NB):
                    ps_log = pslog.tile([128, BN], f32, tag="pslog")
                    if COLPACK:
                        for kp in range(KC_H // 2):
                            nc.tensor.matmul(
                                out=ps_log[0:64, :],
                                lhsT=hT[:, 2 * kp, :],
                                rhs=fcT[:, 2 * kp, n * BN:(n + 1) * BN],
                                start=(kp == 0), stop=(kp == KC_H // 2 - 1),
                                tile_position=(0, 0),
                            )
                            nc.tensor.matmul(
                                out=ps_log[64:128, :],
                                lhsT=hT[:, 2 * kp + 1, :],
                                rhs=fcT[:, 2 * kp + 1, n * BN:(n + 1) * BN],
                                start=(kp == 0), stop=(kp == KC_H // 2 - 1),
                                tile_position=(0, 64),
                            )
                        nc.vector.tensor_tensor(
                            out=logits_sb[:, n, :], in0=ps_log[0:64, :],
                            in1=ps_log[64:128, :], op=mybir.AluOpType.add,
                        )
                    else:
                        for kc in range(KC_H):
                            nc.tensor.matmul(
                                out=ps_log[0:64, :],
                                lhsT=hT[:, kc, :],
                                rhs=fcT[:, kc, n * BN:(n + 1) * BN],
                                start=(kc == 0), stop=(kc == KC_H - 1),
                            )
                        nc.vector.tensor_copy(
                            out=logits_sb[:, n, :], in_=ps_log[0:64, :]
                        )
                    # stream logits out + gumbel in per bank
                    nc.gpsimd.dma_start(
                        out=logits_o.ap()[t, :, n * BN:(n + 1) * BN],
                        in_=logits_sb[:, n, :],
                    )
                    nc.gpsimd.dma_start(
                        out=gum[:, n, :], in_=gum_d.ap()[t, :, n * BN:(n + 1) * BN]
                    )
                    nc.gpsimd.tensor_tensor(
                        out=gum[:, n, :], in0=gum[:, n, :], in1=logits_sb[:, n, :],
                        op=mybir.AluOpType.add,
                    )

                # ---- local top-1 over the vocab shard ----
                m8 = wpool.tile([B, 8], f32, tag="m8")
                i8 = wpool.tile([B, 8], u32, tag="i8")
                gum_flat = gum[:].rearrange("b n c -> b (n c)")
                nc.vector.max(out=m8[:], in_=gum_flat)
                nc.vector.max_index(out=i8[:], in_max=m8[:], in_values=gum_flat)
                cand = wpool.tile([B, 2], f32, tag="cand")
                nc.vector.tensor_copy(out=cand[:, 0:1], in_=m8[:, 0:1])
                nc.vector.tensor_copy(out=cand[:, 1:2], in_=i8[:, 0:1])

                # ---- AllGather candidates, pick global winner ----
                nc.gpsimd.dma_start(out=cb[t].ap(), in_=cand[:])
                nc.gpsimd.collective_compute(
                    "AllGather", mybir.AluOpType.bypass, replica_groups=RG,
                    ins=[cb[t].ap()], outs=[cg[t].ap()],
                )
                cands = wpool.tile([B, 8, 2], f32, tag="cands")
                nc.gpsimd.dma_start(
                    out=cands[:], in_=cg[t].ap().rearrange("(c b) x -> b c x", b=B)
                )
                vals = cands[:, :, 0]
                idxf = cands[:, :, 1]
                red = wpool.tile([B, 12], f32, tag="red")
                nc.vector.tensor_reduce(
                    out=red[:, 0:1], in_=vals, axis=mybir.AxisListType.X,
                    op=mybir.AluOpType.max,
                )
                nc.vector.tensor_copy(out=red[:, 1:9], in_=red[:, 0:1].to_broadcast([B, 8]))
                wi = wpool.tile([B, 8], u32, tag="wi")
                nc.vector.max_index(out=wi[:], in_max=red[:, 1:9], in_values=vals)
                winf = wpool.tile([B, 12], f32, tag="winf")
                nc.vector.tensor_copy(out=winf[:, 0:1], in_=wi[:, 0:1])
                # sel = (iota == winner_core); loc = sum(sel * idx)
                nc.vector.tensor_tensor(
                    out=winf[:, 1:9], in0=iota8f[:],
                    in1=winf[:, 0:1].to_broadcast([B, 8]), op=mybir.AluOpType.is_equal,
                )
                nc.vector.tensor_tensor(
                    out=winf[:, 1:9], in0=winf[:, 1:9], in1=idxf,
                    op=mybir.AluOpType.mult,
                )
                nc.vector.tensor_reduce(
                    out=winf[:, 9:10], in_=winf[:, 1:9], axis=mybir.AxisListType.X,
                    op=mybir.AluOpType.add,
                )
                # glob = 4000 * winner_core + local_idx
                nc.vector.tensor_scalar(
                    out=winf[:, 10:11], in0=winf[:, 0:1], scalar1=float(VS),
                    scalar2=None, op0=mybir.AluOpType.mult,
                )
                nc.vector.tensor_tensor(
                    out=winf[:, 10:11], in0=winf[:, 10:11], in1=winf[:, 9:10],
                    op=mybir.AluOpType.add,
                )
                gidx = wpool.tile([B, 1], u32, tag="gidx")
                nc.vector.tensor_copy(out=gidx[:], in_=winf[:, 10:11])
                nc.gpsimd.dma_start(
                    out=idx_o.ap()[t][:, None], in_=gidx[:].bitcast(i32)
                )

                if t == T - 1:
                    continue
                # ---- gather next embedding, transpose to [D, B] chunks ----
                x_sb = wpool.tile([B, D], f32, tag="x_sb")
                nc.gpsimd.indirect_dma_start(
                    out=x_sb[:], out_offset=None, in_=emb_d.ap(),
                    in_offset=bass.IndirectOffsetOnAxis(ap=gidx[:, :1], axis=0),
                )
                xT = spool.tile([128, KC_D, B], f32, tag="xT")
                ps_x = psx.tile([128, KC_D, B], f32, tag="psx")
                for kc in range(KC_D):
                    nc.tensor.transpose(
                        out=ps_x[:, kc, :], in_=x_sb[:, kc * 128:(kc + 1) * 128],
                        identity=ident[0:64, 0:64],
                    )
                nc.vector.tensor_copy(out=xT[:], in_=ps_x[:])

    nc.compile()
    return nc


@functools.lru_cache(maxsize=1)
def _get_nc():
    return _build_bass()


def _gumbel_cpu():
    """Gumbel noise exactly as jax.random.categorical draws it (CPU backend)."""
    import jax
    import jax.numpy as jnp

    cpu = jax.devices("cpu")[0]
    with jax.default_device(cpu):
        keys = jax.random.split(jax.random.key(42), T)
        fn = jax.jit(lambda k: jax.random.gumbel(k, (B, V), jnp.float32))
        g = np.stack([np.asarray(fn(keys[t])) for t in range(T)])
    return g


def _per_core_inputs(sequence_embedding, emb_table, W_ih, W_hh, b_ih, b_hh,
                     fc_w, fc_b, eos_index, gum):
    seq = np.asarray(sequence_embedding, np.float32)
    emb = np.ascontiguousarray(np.asarray(emb_table, np.float32))
    wih = np.asarray(W_ih, np.float32)
    whh = np.asarray(W_hh, np.float32)
    bih = np.asarray(b_ih, np.float32)
    bhh = np.asarray(b_hh, np.float32)
    fcw = np.asarray(fc_w, np.float32)
    fcb = np.asarray(fc_b, np.float32)
    eos = int(np.asarray(eos_index))

    h0T = np.ascontiguousarray(
        seq.T.reshape(KC_H, 128, B).transpose(1, 0, 2).reshape(128, KC_H * B)
    )
    x0 = emb[eos]  # [D]
    x0T = np.ascontiguousarray(
        np.broadcast_to(
            x0.reshape(KC_D, 128).T[:, :, None], (128, KC_D, B)
        ).reshape(128, KC_D * B)
    )

    in_maps = []
    for c in range(NCORES):
        cs = c * VS
        fcT = np.ascontiguousarray(
            fcw[cs:cs + VS].T.reshape(KC_H, 128, VS).transpose(1, 0, 2)
            .reshape(128, KC_H * VS)
        )
        wih_t = np.empty((128, 3, KC_D, 128), np.float32)
        whh_t = np.empty((128, 3, KC_H, 128), np.float32)
        for g in range(3):
            sl = wih[g * H + c * 128:g * H + (c + 1) * 128, :]  # [128m, D]
            wih_t[:, g] = sl.T.reshape(KC_D, 128, 128).transpose(1, 0, 2)
            sl = whh[g * H + c * 128:g * H + (c + 1) * 128, :]  # [128m, H]
            whh_t[:, g] = sl.T.reshape(KC_H, 128, 128).transpose(1, 0, 2)
        bsum = bih + bhh
        bias = np.stack(
            [
                bsum[0 * H + c * 128:0 * H + (c + 1) * 128],
                bsum[1 * H + c * 128:1 * H + (c + 1) * 128],
                bih[2 * H + c * 128:2 * H + (c + 1) * 128],
                bhh[2 * H + c * 128:2 * H + (c + 1) * 128],
            ],
            axis=1,
        ).astype(np.float32)
        h0s = np.ascontiguousarray(seq[:, c * 128:(c + 1) * 128].T)
        gum_c = np.ascontiguousarray(gum[:, :, cs:cs + VS] + fcb[cs:cs + VS])
        in_maps.append(
            dict(
                fcT=fcT,
                wihT=np.ascontiguousarray(wih_t.reshape(128, 3 * KC_D * 128)),
                whhT=np.ascontiguousarray(whh_t.reshape(128, 3 * KC_H * 128)),
                bias=bias,
                h0T=h0T,
                h0s=h0s,
                x0T=x0T,
                gum=gum_c,
                emb=emb,
            )
        )
    return in_maps


def kernel(sequence_embedding, emb_table, W_ih, W_hh, b_ih, b_hh, fc_w, fc_b,
           eos_index):
    from concourse.bass_utils import run_bass_kernel_spmd

    gum = _gumbel_cpu()
    in_maps = _per_core_inputs(
        sequence_embedding, emb_table, W_ih, W_hh, b_ih, b_hh, fc_w, fc_b,
        eos_index, gum,
    )
    nc = _get_nc()
    res = run_bass_kernel_spmd(nc, in_maps, core_ids=list(range(NCORES)))
    fcb = np.asarray(fc_b, np.float32)
    logits = np.concatenate([r["logits_o"] for r in res.results], axis=2)
    logits += fcb  # exact no-op for fc_b == 0, matches reference otherwise
    indices = res.results[0]["idx_o"].astype(np.int32)
    return indices, logits


# revision 7
# speedup vs baseline: 20.6041x; 20.6041x over previous
"""GRU decoder with categorical sampling on 8 TRN2 NeuronCores.

Sharding: fc_w is vocab-sharded (4000 rows/core) and SBUF-resident; GRU
weights are hidden-sharded (128 h rows/core). Per decode step: each core
computes its gate slice, the h' shards are AllGathered, each core computes
its logits shard against the resident fc_w shard, adds host-precomputed
gumbel noise (bit-exact match of jax.random.categorical's gumbel), takes a
per-shard argmax, AllGathers the 8 (value, index) candidates, reduces to the
global argmax, gathers the next input embedding with an indirect DMA, and
transposes it for the next step's GRU matmuls.
"""

import functools

import numpy as np

T, B, D, H, V = 50, 64, 512, 1024, 32000
NCORES = 8
VS = V // NCORES      # 4000 vocab rows per core
NB = 8                # logits bank count per step
BN = VS // NB         # 500 columns per bank (one PSUM bank)
KC_H = H // 128       # 8 contraction chunks over H
KC_D = D // 128       # 4 contraction chunks over D

COLPACK = True        # pack two K-chunks via tile_position (cols 0-63 / 64-127)


def _build_bass(reps=1):
    import concourse.bass as bass
    import concourse.mybir as mybir
    import concourse.tile as tile
    from concourse import bacc
    from concourse.masks import make_identity

    f32 = mybir.dt.float32
    u32 = mybir.dt.uint32
    i32 = mybir.dt.int32

    nc = bacc.Bacc(num_devices=NCORES)
    RG = [list(range(NCORES))]

    # ---- kernel I/O ----
    fcT_d = nc.declare_dram_parameter("fcT", [128, KC_H * VS], f32, isOutput=False)
    wih_d = nc.declare_dram_parameter("wihT", [128, 3 * KC_D * 128], f32, isOutput=False)
    whh_d = nc.declare_dram_parameter("whhT", [128, 3 * KC_H * 128], f32, isOutput=False)
    bias_d = nc.declare_dram_parameter("bias", [128, 4], f32, isOutput=False)
    h0T_d = nc.declare_dram_parameter("h0T", [128, KC_H * B], f32, isOutput=False)
    h0s_d = nc.declare_dram_parameter("h0s", [128, B], f32, isOutput=False)
    x0T_d = nc.declare_dram_parameter("x0T", [128, KC_D * B], f32, isOutput=False)
    gum_d = nc.declare_dram_parameter("gum", [T, B, VS], f32, isOutput=False)
    emb_d = nc.declare_dram_parameter("emb", [V, D], f32, isOutput=False)

    logits_o = nc.declare_dram_parameter("logits_o", [T, B, VS], f32, isOutput=True)
    idx_o = nc.declare_dram_parameter("idx_o", [T, B], i32, isOutput=True)

    # ---- per-step collective buffers (static, no reuse -> no false deps) ----
    NT = reps * T
    hb = [nc.dram_tensor(f"hb{t}", [128, B], f32) for t in range(NT)]
    hg = [nc.dram_tensor(f"hg{t}", [H, B], f32, addr_space="Shared") for t in range(NT)]
    cb = [nc.dram_tensor(f"cb{t}", [B, 2], f32) for t in range(NT)]
    cg = [nc.dram_tensor(f"cg{t}", [B * NCORES, 2], f32, addr_space="Shared") for t in range(NT)]

    sig = mybir.ActivationFunctionType.Sigmoid
    tanh = mybir.ActivationFunctionType.Tanh

    with tile.TileContext(nc) as tc:
        with (
            tc.tile_pool(name="const", bufs=1) as cpool,
            tc.tile_pool(name="state", bufs=2) as spool,
            tc.tile_pool(name="work", bufs=2) as wpool,
            tc.tile_pool(name="lg", bufs=1) as lgpool,
            tc.tile_pool(name="pslog", bufs=2, space="PSUM") as pslog,
            tc.tile_pool(name="psgh", bufs=2, space="PSUM") as psgh,
            tc.tile_pool(name="psgi", bufs=2, space="PSUM") as psgi,
            tc.tile_pool(name="psx", bufs=2, space="PSUM") as psx,
        ):
            # ---- load constants ----
            fcT = cpool.tile([128, KC_H, VS], f32, tag="fcT")
            for kc in range(KC_H):
                nc.gpsimd.dma_start(
                    out=fcT[:, kc, :], in_=fcT_d.ap()[:, kc * VS:(kc + 1) * VS]
                )
            wih = cpool.tile([128, 3, KC_D, 128], f32, tag="wih")
            nc.gpsimd.dma_start(out=wih[:], in_=wih_d.ap())
            whh = cpool.tile([128, 3, KC_H, 128], f32, tag="whh")
            nc.gpsimd.dma_start(out=whh[:], in_=whh_d.ap())
            bias = cpool.tile([128, 4], f32, tag="bias")
            nc.gpsimd.dma_start(out=bias[:], in_=bias_d.ap())
            ident = cpool.tile([128, 128], f32, tag="ident")
            make_identity(nc, ident[:])
            iota8 = cpool.tile([B, 8], i32, tag="iota8")
            nc.gpsimd.iota(out=iota8[:], pattern=[[1, 8]], base=0, channel_multiplier=0)
            iota8f = cpool.tile([B, 8], f32, tag="iota8f")
            nc.vector.tensor_copy(out=iota8f[:], in_=iota8[:])

            for rep in range(reps):
              hT = spool.tile([128, KC_H, B], f32, tag="hT")
              nc.gpsimd.dma_start(out=hT[:], in_=h0T_d.ap())
              hsh = spool.tile([128, B], f32, tag="hsh")
              nc.gpsimd.dma_start(out=hsh[:], in_=h0s_d.ap())
              xT = spool.tile([128, KC_D, B], f32, tag="xT")
              nc.gpsimd.dma_start(out=xT[:], in_=x0T_d.ap())

              # gh for step 0 (h = h0); later steps compute gh right after
              # the logits matmuls so the PE stays busy during the candidate
              # AllGather + gather window.
              ps_gh = psgh.tile([128, 3, B], f32, tag="psgh")
              for g in range(3):
                  for kc in range(KC_H):
                      nc.tensor.matmul(
                          out=ps_gh[:, g, :],
                          lhsT=whh[:, g, kc, :],
                          rhs=hT[:, kc, :],
                          start=(kc == 0),
                          stop=(kc == KC_H - 1),
                      )

              for tt in range(T):
                t = rep * T + tt
                # ---- GRU: h(t+1) = GRUCell(x_t, h_t); gh precomputed ----
                ps_gi = psgi.tile([128, 3, B], f32, tag="psgi")
                for g in range(3):
                    for kc in range(KC_D):
                        nc.tensor.matmul(
                            out=ps_gi[:, g, :],
                            lhsT=wih[:, g, kc, :],
                            rhs=xT[:, kc, :],
                            start=(kc == 0),
                            stop=(kc == KC_D - 1),
                        )

                gt = wpool.tile([128, 6, B], f32, tag="gates")
                ghs = wpool.tile([128, 3, B], f32, tag="ghs")
                nc.scalar.copy(out=ghs[:], in_=ps_gh[:])
                # r = sigmoid(gi_r + gh_r + (b_ih_r + b_hh_r))
                nc.vector.tensor_tensor(
                    out=gt[:, 0, :], in0=ps_gi[:, 0, :], in1=ghs[:, 0, :],
                    op=mybir.AluOpType.add,
                )
                nc.scalar.activation(out=gt[:, 1, :], in_=gt[:, 0, :], func=sig,
                                     bias=bias[:, 0:1])
                # z = sigmoid(gi_z + gh_z + (b_ih_z + b_hh_z))
                nc.vector.tensor_tensor(
                    out=gt[:, 2, :], in0=ps_gi[:, 1, :], in1=ghs[:, 1, :],
                    op=mybir.AluOpType.add,
                )
                nc.scalar.activation(out=gt[:, 3, :], in_=gt[:, 2, :], func=sig,
                                     bias=bias[:, 1:2])
                # n = tanh(gi_n + b_ih_n + r * (gh_n + b_hh_n))
                nc.vector.tensor_scalar_add(out=gt[:, 4, :], in0=ghs[:, 2, :],
                                            scalar1=bias[:, 3:4])
                nc.vector.tensor_tensor(
                    out=gt[:, 4, :], in0=gt[:, 1, :], in1=gt[:, 4, :],
                    op=mybir.AluOpType.mult,
                )
                nc.vector.tensor_tensor(
                    out=gt[:, 4, :], in0=ps_gi[:, 2, :], in1=gt[:, 4, :],
                    op=mybir.AluOpType.add,
                )
                nc.scalar.activation(out=gt[:, 5, :], in_=gt[:, 4, :], func=tanh,
                                     bias=bias[:, 2:3])
                # h' = (1 - z) * n + z * h
                hsh_new = spool.tile([128, B], f32, tag="hsh")
                nc.vector.tensor_scalar(
                    out=gt[:, 0, :], in0=gt[:, 3, :], scalar1=-1.0, scalar2=1.0,
                    op0=mybir.AluOpType.mult, op1=mybir.AluOpType.add,
                )
                nc.vector.tensor_tensor(
                    out=gt[:, 0, :], in0=gt[:, 0, :], in1=gt[:, 5, :],
                    op=mybir.AluOpType.mult,
                )
                nc.vector.tensor_tensor(
                    out=gt[:, 2, :], in0=gt[:, 3, :], in1=hsh[:],
                    op=mybir.AluOpType.mult,
                )
                nc.vector.tensor_tensor(
                    out=hsh_new[:], in0=gt[:, 0, :], in1=gt[:, 2, :],
                    op=mybir.AluOpType.add,
                )
                hsh = hsh_new

                # ---- AllGather h' shards -> full h(t+1), transposed layout ----
                nc.gpsimd.dma_start(out=hb[t].ap(), in_=hsh[:])
                nc.gpsimd.collective_compute(
                    "AllGather", mybir.AluOpType.bypass, replica_groups=RG,
                    ins=[hb[t].ap()], outs=[hg[t].ap()],
                )
                hT = spool.tile([128, KC_H, B], f32, tag="hT")
                nc.gpsimd.dma_start(
                    out=hT[:], in_=hg[t].ap().rearrange("(c p) b -> p c b", p=128)
                )

                # ---- logits_t = h(t+1) @ fc_w_shard.T  (+ gumbel, sampled) ----
                gum = lgpool.tile([B, NB, BN], f32, tag="gum")
                for n in range(NB):
                    nc.gpsimd.dma_start(
                        out=gum[:, n, :], in_=gum_d.ap()[tt, :, n * BN:(n + 1) * BN]
                    )
                bankv = wpool.tile([B, 8], f32, tag="bankv")
                banki = wpool.tile([B, 8], f32, tag="banki")
                ps_logs = []
                for n in range(NB):
                    ps_log = pslog.tile([128, BN], f32, tag="pslog")
                    ps_logs.append(ps_log)
                    if COLPACK:
                        for kp in range(KC_H // 2):
                            nc.tensor.matmul(
                                out=ps_log[0:64, :],
                                lhsT=hT[:, 2 * kp, :],
                                rhs=fcT[:, 2 * kp, n * BN:(n + 1) * BN],
                                start=(kp == 0), stop=(kp == KC_H // 2 - 1),
                                tile_position=(0, 0),
                            )
                            nc.tensor.matmul(
                                out=ps_log[64:128, :],
                                lhsT=hT[:, 2 * kp + 1, :],
                                rhs=fcT[:, 2 * kp + 1, n * BN:(n + 1) * BN],
                                start=(kp == 0), stop=(kp == KC_H // 2 - 1),
                                tile_position=(0, 64),
                            )
                    else:
                        for kc in range(KC_H):
                            nc.tensor.matmul(
                                out=ps_log[0:64, :],
                                lhsT=hT[:, kc, :],
                                rhs=fcT[:, kc, n * BN:(n + 1) * BN],
                                start=(kc == 0), stop=(kc == KC_H - 1),
                            )
                # gh for the next step: emitted here so the PE runs it during
                # the sampling/AllGather/gather window (its input is ready).
                if tt < T - 1:
                    ps_gh = psgh.tile([128, 3, B], f32, tag="psgh")
                    for g in range(3):
                        for kc in range(KC_H):
                            nc.tensor.matmul(
                                out=ps_gh[:, g, :],
                                lhsT=whh[:, g, kc, :],
                                rhs=hT[:, kc, :],
                                start=(kc == 0),
                                stop=(kc == KC_H - 1),
                            )
                # per-bank epilogue: merge halves (pure logits out) + gumbel
                # add + per-bank top-1, pipelined behind later banks' matmuls
                for n in range(NB):
                    ps_log = ps_logs[n]
                    lsc = wpool.tile([B, BN], f32, tag="lsc")
                    if COLPACK:
                        # TensorTensor may read only one PSUM operand: stage
                        # the upper half through SBUF on the scalar engine.
                        nc.scalar.copy(out=lsc[:], in_=ps_log[64:128, :])
                        nc.vector.tensor_tensor(
                            out=lsc[:], in0=ps_log[0:64, :], in1=lsc[:],
                            op=mybir.AluOpType.add,
                        )
                    else:
                        nc.vector.tensor_copy(out=lsc[:], in_=ps_log[0:64, :])
                    nc.gpsimd.dma_start(
                        out=logits_o.ap()[tt, :, n * BN:(n + 1) * BN], in_=lsc[:],
                    )
                    lg = wpool.tile([B, BN], f32, tag="lgscr")
                    nc.vector.tensor_tensor(
                        out=lg[:], in0=gum[:, n, :], in1=lsc[:],
                        op=mybir.AluOpType.add,
                    )
                    m8 = wpool.tile([B, 8], f32, tag="m8")
                    i8 = wpool.tile([B, 8], u32, tag="i8")
                    nc.vector.max(out=m8[:], in_=lg[:])
                    nc.vector.max_index(out=i8[:], in_max=m8[:], in_values=lg[:])
                    nc.vector.tensor_copy(out=bankv[:, n:n + 1], in_=m8[:, 0:1])
                    nc.vector.tensor_copy(out=banki[:, n:n + 1], in_=i8[:, 0:1])

                # ---- combine the 8 banks into the shard top-1 ----
                shred = wpool.tile([B, 12], f32, tag="shred")
                nc.vector.tensor_reduce(
                    out=shred[:, 0:1], in_=bankv[:], axis=mybir.AxisListType.X,
                    op=mybir.AluOpType.max,
                )
                nc.vector.tensor_copy(
                    out=shred[:, 1:9], in_=shred[:, 0:1].to_broadcast([B, 8])
                )
                wb = wpool.tile([B, 8], u32, tag="wb")
                nc.vector.max_index(out=wb[:], in_max=shred[:, 1:9], in_values=bankv[:])
                wbf = wpool.tile([B, 4], f32, tag="wbf")
                nc.vector.tensor_copy(out=wbf[:, 0:1], in_=wb[:, 0:1])
                nc.vector.tensor_tensor(
                    out=shred[:, 1:9], in0=iota8f[:],
                    in1=wbf[:, 0:1].to_broadcast([B, 8]), op=mybir.AluOpType.is_equal,
                )
                nc.vector.tensor_tensor(
                    out=shred[:, 1:9], in0=shred[:, 1:9], in1=banki[:],
                    op=mybir.AluOpType.mult,
                )
                nc.vector.tensor_reduce(
                    out=shred[:, 9:10], in_=shred[:, 1:9], axis=mybir.AxisListType.X,
                    op=mybir.AluOpType.add,
                )
                nc.vector.tensor_scalar(
                    out=shred[:, 10:11], in0=wbf[:, 0:1], scalar1=float(BN),
                    scalar2=None, op0=mybir.AluOpType.mult,
                )
                cand = wpool.tile([B, 2], f32, tag="cand")
                nc.vector.tensor_copy(out=cand[:, 0:1], in_=shred[:, 0:1])
                nc.vector.tensor_tensor(
                    out=cand[:, 1:2], in0=shred[:, 10:11], in1=shred[:, 9:10],
                    op=mybir.AluOpType.add,
                )

                # ---- AllGather candidates, pick global winner ----
                nc.gpsimd.dma_start(out=cb[t].ap(), in_=cand[:])
                nc.gpsimd.collective_compute(
                    "AllGather", mybir.AluOpType.bypass, replica_groups=RG,
                    ins=[cb[t].ap()], outs=[cg[t].ap()],
                )
                cands = wpool.tile([B, 8, 2], f32, tag="cands")
                nc.gpsimd.dma_start(
                    out=cands[:], in_=cg[t].ap().rearrange("(c b) x -> b c x", b=B)
                )
                vals = cands[:, :, 0]
                idxf = cands[:, :, 1]
                red = wpool.tile([B, 12], f32, tag="red")
                nc.vector.tensor_reduce(
                    out=red[:, 0:1], in_=vals, axis=mybir.AxisListType.X,
                    op=mybir.AluOpType.max,
                )
                nc.vector.tensor_copy(out=red[:, 1:9], in_=red[:, 0:1].to_broadcast([B, 8]))
                wi = wpool.tile([B, 8], u32, tag="wi")
                nc.vector.max_index(out=wi[:], in_max=red[:, 1:9], in_values=vals)
                winf = wpool.tile([B, 12], f32, tag="winf")
                nc.vector.tensor_copy(out=winf[:, 0:1], in_=wi[:, 0:1])
                # sel = (iota == winner_core); loc = sum(sel * idx)
                nc.vector.tensor_tensor(
                    out=winf[:, 1:9], in0=iota8f[:],
                    in1=winf[:, 0:1].to_broadcast([B, 8]), op=mybir.AluOpType.is_equal,
                )
                nc.vector.tensor_tensor(
                    out=winf[:, 1:9], in0=winf[:, 1:9], in1=idxf,
                    op=mybir.AluOpType.mult,
                )
                nc.vector.tensor_reduce(
                    out=winf[:, 9:10], in_=winf[:, 1:9], axis=mybir.AxisListType.X,
                    op=mybir.AluOpType.add,
                )
                # glob = 4000 * winner_core + local_idx
                nc.vector.tensor_scalar(
                    out=winf[:, 10:11], in0=winf[:, 0:1], scalar1=float(VS),
                    scalar2=None, op0=mybir.AluOpType.mult,
                )
                nc.vector.tensor_tensor(
                    out=winf[:, 10:11], in0=winf[:, 10:11], in1=winf[:, 9:10],
                    op=mybir.AluOpType.add,
                )
                gidx = wpool.tile([B, 1], u32, tag="gidx")
                nc.vector.tensor_copy(out=gidx[:], in_=winf[:, 10:11])
                nc.gpsimd.dma_start(
                    out=idx_o.ap()[tt][:, None], in_=gidx[:].bitcast(i32)
                )

                if tt == T - 1:
                    continue
                # ---- gather next embedding, transpose to [D, B] chunks ----
                x_sb = wpool.tile([B, D], f32, tag="x_sb")
                nc.gpsimd.indirect_dma_start(
                    out=x_sb[:], out_offset=None, in_=emb_d.ap(),
                    in_offset=bass.IndirectOffsetOnAxis(ap=gidx[:, :1], axis=0),
                )
                xT = spool.tile([128, KC_D, B], f32, tag="xT")
                ps_x = psx.tile([128, KC_D, B], f32, tag="psx")
                for kc in range(KC_D):
                    nc.tensor.transpose(
                        out=ps_x[:, kc, :], in_=x_sb[:, kc * 128:(kc + 1) * 128],
                        identity=ident[0:64, 0:64],
                    )
                nc.vector.tensor_copy(out=xT[:], in_=ps_x[:])

    nc.compile()
    return nc


@functools.lru_cache(maxsize=4)
def _get_nc(reps=1):
    return _build_bass(reps)


def _gumbel_cpu():
    """Gumbel noise exactly as jax.random.categorical draws it (CPU backend)."""
    import jax
    import jax.numpy as jnp

    cpu = jax.devices("cpu")[0]
    with jax.default_device(cpu):
        keys = jax.random.split(jax.random.key(42), T)
        fn = jax.jit(lambda k: jax.random.gumbel(k, (B, V), jnp.float32))
        g = np.stack([np.asarray(fn(keys[t])) for t in range(T)])
    return g


def _per_core_inputs(sequence_embedding, emb_table, W_ih, W_hh, b_ih, b_hh,
                     fc_w, fc_b, eos_index, gum):
    seq = np.asarray(sequence_embedding, np.float32)
    emb = np.ascontiguousarray(np.asarray(emb_table, np.float32))
    wih = np.asarray(W_ih, np.float32)
    whh = np.asarray(W_hh, np.float32)
    bih = np.asarray(b_ih, np.float32)
    bhh = np.asarray(b_hh, np.float32)
    fcw = np.asarray(fc_w, np.float32)
    fcb = np.asarray(fc_b, np.float32)
    eos = int(np.asarray(eos_index))

    h0T = np.ascontiguousarray(
        seq.T.reshape(KC_H, 128, B).transpose(1, 0, 2).reshape(128, KC_H * B)
    )
    x0 = emb[eos]  # [D]
    x0T = np.ascontiguousarray(
        np.broadcast_to(
            x0.reshape(KC_D, 128).T[:, :, None], (128, KC_D, B)
        ).reshape(128, KC_D * B)
    )

    in_maps = []
    for c in range(NCORES):
        cs = c * VS
        fcT = np.ascontiguousarray(
            fcw[cs:cs + VS].T.reshape(KC_H, 128, VS).transpose(1, 0, 2)
            .reshape(128, KC_H * VS)
        )
        wih_t = np.empty((128, 3, KC_D, 128), np.float32)
        whh_t = np.empty((128, 3, KC_H, 128), np.float32)
        for g in range(3):
            sl = wih[g * H + c * 128:g * H + (c + 1) * 128, :]  # [128m, D]
            wih_t[:, g] = sl.T.reshape(KC_D, 128, 128).transpose(1, 0, 2)
            sl = whh[g * H + c * 128:g * H + (c + 1) * 128, :]  # [128m, H]
            whh_t[:, g] = sl.T.reshape(KC_H, 128, 128).transpose(1, 0, 2)
        bsum = bih + bhh
        bias = np.stack(
            [
                bsum[0 * H + c * 128:0 * H + (c + 1) * 128],
                bsum[1 * H + c * 128:1 * H + (c + 1) * 128],
                bih[2 * H + c * 128:2 * H + (c + 1) * 128],
                bhh[2 * H + c * 128:2 * H + (c + 1) * 128],
            ],
            axis=1,
        ).astype(np.float32)
        h0s = np.ascontiguousarray(seq[:, c * 128:(c + 1) * 128].T)
        gum_c = np.ascontiguousarray(gum[:, :, cs:cs + VS] + fcb[cs:cs + VS])
        in_maps.append(
            dict(
                fcT=fcT,
                wihT=np.ascontiguousarray(wih_t.reshape(128, 3 * KC_D * 128)),
                whhT=np.ascontiguousarray(whh_t.reshape(128, 3 * KC_H * 128)),
                bias=bias,
                h0T=h0T,
                h0s=h0s,
                x0T=x0T,
                gum=gum_c,
                emb=emb,
            )
        )
    return in_maps


def kernel(sequence_embedding, emb_table, W_ih, W_hh, b_ih, b_hh, fc_w, fc_b,
           eos_index):
    from concourse.bass_utils import run_bass_kernel_spmd

    gum = _gumbel_cpu()
    in_maps = _per_core_inputs(
        sequence_embedding, emb_table, W_ih, W_hh, b_ih, b_hh, fc_w, fc_b,
        eos_index, gum,
    )
    nc = _get_nc()
    res = run_bass_kernel_spmd(nc, in_maps, core_ids=list(range(NCORES)))
    fcb = np.asarray(fc_b, np.float32)
    logits = np.concatenate([r["logits_o"] for r in res.results], axis=2)
    logits += fcb  # exact no-op for fc_b == 0, matches reference otherwise
    indices = res.results[0]["idx_o"].astype(np.int32)
    return indices, logits


# revision 11
# speedup vs baseline: 21.3436x; 1.0359x over previous
"""GRU decoder with categorical sampling on 8 TRN2 NeuronCores.

Sharding: fc_w is vocab-sharded (4000 rows/core) and SBUF-resident; GRU
weights are hidden-sharded (128 h rows/core). Per decode step: each core
computes its gate slice, the h' shards are AllGathered, each core computes
its logits shard against the resident fc_w shard, adds host-precomputed
gumbel noise (bit-exact match of jax.random.categorical's gumbel), takes a
per-shard argmax, AllGathers the 8 (value, index) candidates, reduces to the
global argmax, gathers the next input embedding with an indirect DMA, and
transposes it for the next step's GRU matmuls.
"""

import functools

import numpy as np

T, B, D, H, V = 50, 64, 512, 1024, 32000
NCORES = 8
VS = V // NCORES      # 4000 vocab rows per core
NB = 8                # logits bank count per step
BN = VS // NB         # 500 columns per bank (one PSUM bank)
KC_H = H // 128       # 8 contraction chunks over H
KC_D = D // 128       # 4 contraction chunks over D

COLPACK = True        # pack two K-chunks via tile_position (cols 0-63 / 64-127)


def _build_bass(reps=1):
    import concourse.bass as bass
    import concourse.mybir as mybir
    import concourse.tile as tile
    from concourse import bacc
    from concourse.masks import make_identity

    f32 = mybir.dt.float32
    u32 = mybir.dt.uint32
    i32 = mybir.dt.int32

    nc = bacc.Bacc(num_devices=NCORES)
    RG = [list(range(NCORES))]

    # ---- kernel I/O ----
    fcT_d = nc.declare_dram_parameter("fcT", [128, KC_H * VS], f32, isOutput=False)
    wih_d = nc.declare_dram_parameter("wihT", [128, 3 * KC_D * 128], f32, isOutput=False)
    whh_d = nc.declare_dram_parameter("whhT", [128, 3 * KC_H * 128], f32, isOutput=False)
    bias_d = nc.declare_dram_parameter("bias", [128, 4], f32, isOutput=False)
    h0T_d = nc.declare_dram_parameter("h0T", [128, KC_H * B], f32, isOutput=False)
    h0s_d = nc.declare_dram_parameter("h0s", [128, B], f32, isOutput=False)
    x0T_d = nc.declare_dram_parameter("x0T", [128, KC_D * B], f32, isOutput=False)
    gum_d = nc.declare_dram_parameter("gum", [T, B, VS], f32, isOutput=False)
    emb_d = nc.declare_dram_parameter("emb", [V, D], f32, isOutput=False)

    logits_o = nc.declare_dram_parameter("logits_o", [T, B, VS], f32, isOutput=True)
    idx_o = nc.declare_dram_parameter("idx_o", [T, B], i32, isOutput=True)

    # ---- per-step collective buffers (static, no reuse -> no false deps) ----
    NT = reps * T
    hb = [nc.dram_tensor(f"hb{t}", [128, B], f32) for t in range(NT)]
    hg = [nc.dram_tensor(f"hg{t}", [H, B], f32, addr_space="Shared") for t in range(NT)]
    cb = [nc.dram_tensor(f"cb{t}", [B, 2], f32) for t in range(NT)]
    cg = [nc.dram_tensor(f"cg{t}", [B * NCORES, 2], f32, addr_space="Shared") for t in range(NT)]

    sig = mybir.ActivationFunctionType.Sigmoid
    tanh = mybir.ActivationFunctionType.Tanh

    with tile.TileContext(nc) as tc:
        with (
            tc.tile_pool(name="const", bufs=1) as cpool,
            tc.tile_pool(name="state", bufs=2) as spool,
            tc.tile_pool(name="work", bufs=2) as wpool,
            tc.tile_pool(name="lg", bufs=1) as lgpool,
            tc.tile_pool(name="pslog", bufs=2, space="PSUM") as pslog,
            tc.tile_pool(name="psgh", bufs=2, space="PSUM") as psgh,
            tc.tile_pool(name="psgi", bufs=2, space="PSUM") as psgi,
            tc.tile_pool(name="psx", bufs=2, space="PSUM") as psx,
        ):
            # ---- load constants ----
            fcT = cpool.tile([128, KC_H, VS], f32, tag="fcT")
            for kc in range(KC_H):
                nc.gpsimd.dma_start(
                    out=fcT[:, kc, :], in_=fcT_d.ap()[:, kc * VS:(kc + 1) * VS]
                )
            wih = cpool.tile([128, 3, KC_D, 128], f32, tag="wih")
            nc.gpsimd.dma_start(out=wih[:], in_=wih_d.ap())
            whh = cpool.tile([128, 3, KC_H, 128], f32, tag="whh")
            nc.gpsimd.dma_start(out=whh[:], in_=whh_d.ap())
            bias = cpool.tile([128, 4], f32, tag="bias")
            nc.gpsimd.dma_start(out=bias[:], in_=bias_d.ap())
            ident = cpool.tile([128, 128], f32, tag="ident")
            make_identity(nc, ident[:])
            iota8 = cpool.tile([B, 8], i32, tag="iota8")
            nc.gpsimd.iota(out=iota8[:], pattern=[[1, 8]], base=0, channel_multiplier=0)
            iota8f = cpool.tile([B, 8], f32, tag="iota8f")
            nc.vector.tensor_copy(out=iota8f[:], in_=iota8[:])

            for rep in range(reps):
              hT = spool.tile([128, KC_H, B], f32, tag="hT")
              nc.gpsimd.dma_start(out=hT[:], in_=h0T_d.ap())
              hsh = spool.tile([128, B], f32, tag="hsh")
              nc.gpsimd.dma_start(out=hsh[:], in_=h0s_d.ap())
              xT = spool.tile([128, KC_D, B], f32, tag="xT")
              nc.gpsimd.dma_start(out=xT[:], in_=x0T_d.ap())

              for tt in range(T):
                t = rep * T + tt
                # ---- GRU: h(t+1) = GRUCell(x_t, h_t) ----
                ps_gh = psgh.tile([128, 3, B], f32, tag="psgh")
                for g in range(3):
                    for kc in range(KC_H):
                        nc.tensor.matmul(
                            out=ps_gh[:, g, :],
                            lhsT=whh[:, g, kc, :],
                            rhs=hT[:, kc, :],
                            start=(kc == 0),
                            stop=(kc == KC_H - 1),
                        )
                ps_gi = psgi.tile([128, 3, B], f32, tag="psgi")
                for g in range(3):
                    for kc in range(KC_D):
                        nc.tensor.matmul(
                            out=ps_gi[:, g, :],
                            lhsT=wih[:, g, kc, :],
                            rhs=xT[:, kc, :],
                            start=(kc == 0),
                            stop=(kc == KC_D - 1),
                        )

                gt = wpool.tile([128, 6, B], f32, tag="gates")
                ghs = wpool.tile([128, 3, B], f32, tag="ghs")
                nc.scalar.copy(out=ghs[:], in_=ps_gh[:])
                # r = sigmoid(gi_r + gh_r + (b_ih_r + b_hh_r))
                nc.vector.tensor_tensor(
                    out=gt[:, 0, :], in0=ps_gi[:, 0, :], in1=ghs[:, 0, :],
                    op=mybir.AluOpType.add,
                )
                nc.scalar.activation(out=gt[:, 1, :], in_=gt[:, 0, :], func=sig,
                                     bias=bias[:, 0:1])
                # z = sigmoid(gi_z + gh_z + (b_ih_z + b_hh_z))
                nc.vector.tensor_tensor(
                    out=gt[:, 2, :], in0=ps_gi[:, 1, :], in1=ghs[:, 1, :],
                    op=mybir.AluOpType.add,
                )
                nc.scalar.activation(out=gt[:, 3, :], in_=gt[:, 2, :], func=sig,
                                     bias=bias[:, 1:2])
                # n = tanh(gi_n + b_ih_n + r * (gh_n + b_hh_n))
                nc.vector.tensor_scalar_add(out=gt[:, 4, :], in0=ghs[:, 2, :],
                                            scalar1=bias[:, 3:4])
                nc.vector.tensor_tensor(
                    out=gt[:, 4, :], in0=gt[:, 1, :], in1=gt[:, 4, :],
                    op=mybir.AluOpType.mult,
                )
                nc.vector.tensor_tensor(
                    out=gt[:, 4, :], in0=ps_gi[:, 2, :], in1=gt[:, 4, :],
                    op=mybir.AluOpType.add,
                )
                nc.scalar.activation(out=gt[:, 5, :], in_=gt[:, 4, :], func=tanh,
                                     bias=bias[:, 2:3])
                # h' = (1 - z) * n + z * h
                hsh_new = spool.tile([128, B], f32, tag="hsh")
                nc.vector.tensor_scalar(
                    out=gt[:, 0, :], in0=gt[:, 3, :], scalar1=-1.0, scalar2=1.0,
                    op0=mybir.AluOpType.mult, op1=mybir.AluOpType.add,
                )
                nc.vector.tensor_tensor(
                    out=gt[:, 0, :], in0=gt[:, 0, :], in1=gt[:, 5, :],
                    op=mybir.AluOpType.mult,
                )
                nc.vector.tensor_tensor(
                    out=gt[:, 2, :], in0=gt[:, 3, :], in1=hsh[:],
                    op=mybir.AluOpType.mult,
                )
                nc.vector.tensor_tensor(
                    out=hsh_new[:], in0=gt[:, 0, :], in1=gt[:, 2, :],
                    op=mybir.AluOpType.add,
                )
                hsh = hsh_new

                # ---- AllGather h' shards -> full h(t+1), transposed layout ----
                nc.gpsimd.dma_start(out=hb[t].ap(), in_=hsh[:])
                nc.gpsimd.collective_compute(
                    "AllGather", mybir.AluOpType.bypass, replica_groups=RG,
                    ins=[hb[t].ap()], outs=[hg[t].ap()],
                )
                hT = spool.tile([128, KC_H, B], f32, tag="hT")
                nc.gpsimd.dma_start(
                    out=hT[:], in_=hg[t].ap().rearrange("(c p) b -> p c b", p=128)
                )

                # ---- logits_t = h(t+1) @ fc_w_shard.T  (+ gumbel, sampled) ----
                logits_sb = lgpool.tile([B, NB, BN], f32, tag="logits")
                gum = lgpool.tile([B, NB, BN], f32, tag="gum")
                for n in range(NB):
                    ps_log = pslog.tile([128, BN], f32, tag="pslog")
                    if COLPACK:
                        for kp in range(KC_H // 2):
                            nc.tensor.matmul(
                                out=ps_log[0:64, :],
                                lhsT=hT[:, 2 * kp, :],
                                rhs=fcT[:, 2 * kp, n * BN:(n + 1) * BN],
                                start=(kp == 0), stop=(kp == KC_H // 2 - 1),
                                tile_position=(0, 0),
                            )
                            nc.tensor.matmul(
                                out=ps_log[64:128, :],
                                lhsT=hT[:, 2 * kp + 1, :],
                                rhs=fcT[:, 2 * kp + 1, n * BN:(n + 1) * BN],
                                start=(kp == 0), stop=(kp == KC_H // 2 - 1),
                                tile_position=(0, 64),
                            )
                        # TensorTensor may read only one PSUM operand: stage
                        # the upper half through SBUF on the scalar engine.
                        nc.scalar.copy(
                            out=logits_sb[:, n, :], in_=ps_log[64:128, :]
                        )
                        nc.vector.tensor_tensor(
                            out=logits_sb[:, n, :], in0=ps_log[0:64, :],
                            in1=logits_sb[:, n, :], op=mybir.AluOpType.add,
                        )
                    else:
                        for kc in range(KC_H):
                            nc.tensor.matmul(
                                out=ps_log[0:64, :],
                                lhsT=hT[:, kc, :],
                                rhs=fcT[:, kc, n * BN:(n + 1) * BN],
                                start=(kc == 0), stop=(kc == KC_H - 1),
                            )
                        nc.vector.tensor_copy(
                            out=logits_sb[:, n, :], in_=ps_log[0:64, :]
                        )
                    # stream logits out + gumbel in per bank
                    nc.gpsimd.dma_start(
                        out=logits_o.ap()[tt, :, n * BN:(n + 1) * BN],
                        in_=logits_sb[:, n, :],
                    )
                    nc.gpsimd.dma_start(
                        out=gum[:, n, :], in_=gum_d.ap()[tt, :, n * BN:(n + 1) * BN]
                    )
                    nc.gpsimd.tensor_tensor(
                        out=gum[:, n, :], in0=gum[:, n, :], in1=logits_sb[:, n, :],
                        op=mybir.AluOpType.add,
                    )

                # ---- local top-1 over the vocab shard ----
                m8 = wpool.tile([B, 8], f32, tag="m8")
                i8 = wpool.tile([B, 8], u32, tag="i8")
                gum_flat = gum[:].rearrange("b n c -> b (n c)")
                nc.vector.max(out=m8[:], in_=gum_flat)
                nc.vector.max_index(out=i8[:], in_max=m8[:], in_values=gum_flat)
                cand = wpool.tile([B, 2], f32, tag="cand")
                nc.vector.tensor_copy(out=cand[:, 0:1], in_=m8[:, 0:1])
                nc.vector.tensor_copy(out=cand[:, 1:2], in_=i8[:, 0:1])

                # ---- AllGather candidates, pick global winner ----
                nc.gpsimd.dma_start(out=cb[t].ap(), in_=cand[:])
                nc.gpsimd.collective_compute(
                    "AllGather", mybir.AluOpType.bypass, replica_groups=RG,
                    ins=[cb[t].ap()], outs=[cg[t].ap()],
                )
                cands = wpool.tile([B, 8, 2], f32, tag="cands")
                nc.gpsimd.dma_start(
                    out=cands[:], in_=cg[t].ap().rearrange("(c b) x -> b c x", b=B)
                )
                vals = cands[:, :, 0]
                idxf = cands[:, :, 1]
                red = wpool.tile([B, 12], f32, tag="red")
                nc.vector.tensor_reduce(
                    out=red[:, 0:1], in_=vals, axis=mybir.AxisListType.X,
                    op=mybir.AluOpType.max,
                )
                nc.vector.tensor_copy(out=red[:, 1:9], in_=red[:, 0:1].to_broadcast([B, 8]))
                wi = wpool.tile([B, 8], u32, tag="wi")
                nc.vector.max_index(out=wi[:], in_max=red[:, 1:9], in_values=vals)
                winf = wpool.tile([B, 12], f32, tag="winf")
                nc.vector.tensor_copy(out=winf[:, 0:1], in_=wi[:, 0:1])
                # sel = (iota == winner_core); loc = sum(sel * idx)
                nc.vector.tensor_tensor(
                    out=winf[:, 1:9], in0=iota8f[:],
                    in1=winf[:, 0:1].to_broadcast([B, 8]), op=mybir.AluOpType.is_equal,
                )
                nc.vector.tensor_tensor(
                    out=winf[:, 1:9], in0=winf[:, 1:9], in1=idxf,
                    op=mybir.AluOpType.mult,
                )
                nc.vector.tensor_reduce(
                    out=winf[:, 9:10], in_=winf[:, 1:9], axis=mybir.AxisListType.X,
                    op=mybir.AluOpType.add,
                )
                # glob = 4000 * winner_core + local_idx
                nc.vector.tensor_scalar(
                    out=winf[:, 10:11], in0=winf[:, 0:1], scalar1=float(VS),
                    scalar2=None, op0=mybir.AluOpType.mult,
                )
                nc.vector.tensor_tensor(
                    out=winf[:, 10:11], in0=winf[:, 10:11], in1=winf[:, 9:10],
                    op=mybir.AluOpType.add,
                )
                gidx = wpool.tile([B, 1], u32, tag="gidx")
                nc.vector.tensor_copy(out=gidx[:], in_=winf[:, 10:11])
                nc.gpsimd.dma_start(
                    out=idx_o.ap()[tt][:, None], in_=gidx[:].bitcast(i32)
                )

                if tt == T - 1:
                    continue
                # ---- gather next embedding, transpose to [D, B] chunks ----
                x_sb = wpool.tile([B, D], f32, tag="x_sb")
                nc.gpsimd.indirect_dma_start(
                    out=x_sb[:], out_offset=None, in_=emb_d.ap(),
                    in_offset=bass.IndirectOffsetOnAxis(ap=gidx[:, :1], axis=0),
                )
                xT = spool.tile([128, KC_D, B], f32, tag="xT")
                ps_x = psx.tile([128, KC_D, B], f32, tag="psx")
                for kc in range(KC_D):
                    nc.tensor.transpose(
                        out=ps_x[:, kc, :], in_=x_sb[:, kc * 128:(kc + 1) * 128],
                        identity=ident[0:64, 0:64],
                    )
                nc.vector.tensor_copy(out=xT[:], in_=ps_x[:])

    nc.compile()
    return nc


@functools.lru_cache(maxsize=4)
def _get_nc(reps=1):
    return _build_bass(reps)


def _gumbel_cpu():
    """Gumbel noise exactly as jax.random.categorical draws it (CPU backend)."""
    import jax
    import jax.numpy as jnp

    cpu = jax.devices("cpu")[0]
    with jax.default_device(cpu):
        keys = jax.random.split(jax.random.key(42), T)
        fn = jax.jit(lambda k: jax.random.gumbel(k, (B, V), jnp.float32))
        g = np.stack([np.asarray(fn(keys[t])) for t in range(T)])
    return g


def _per_core_inputs(sequence_embedding, emb_table, W_ih, W_hh, b_ih, b_hh,
                     fc_w, fc_b, eos_index, gum):
    seq = np.asarray(sequence_embedding, np.float32)
    emb = np.ascontiguousarray(np.asarray(emb_table, np.float32))
    wih = np.asarray(W_ih, np.float32)
    whh = np.asarray(W_hh, np.float32)
    bih = np.asarray(b_ih, np.float32)
    bhh = np.asarray(b_hh, np.float32)
    fcw = np.asarray(fc_w, np.float32)
    fcb = np.asarray(fc_b, np.float32)
    eos = int(np.asarray(eos_index))

    h0T = np.ascontiguousarray(
        seq.T.reshape(KC_H, 128, B).transpose(1, 0, 2).reshape(128, KC_H * B)
    )
    x0 = emb[eos]  # [D]
    x0T = np.ascontiguousarray(
        np.broadcast_to(
            x0.reshape(KC_D, 128).T[:, :, None], (128, KC_D, B)
        ).reshape(128, KC_D * B)
    )

    in_maps = []
    for c in range(NCORES):
        cs = c * VS
        fcT = np.ascontiguousarray(
            fcw[cs:cs + VS].T.reshape(KC_H, 128, VS).transpose(1, 0, 2)
            .reshape(128, KC_H * VS)
        )
        wih_t = np.empty((128, 3, KC_D, 128), np.float32)
        whh_t = np.empty((128, 3, KC_H, 128), np.float32)
        for g in range(3):
            sl = wih[g * H + c * 128:g * H + (c + 1) * 128, :]  # [128m, D]
            wih_t[:, g] = sl.T.reshape(KC_D, 128, 128).transpose(1, 0, 2)
            sl = whh[g * H + c * 128:g * H + (c + 1) * 128, :]  # [128m, H]
            whh_t[:, g] = sl.T.reshape(KC_H, 128, 128).transpose(1, 0, 2)
        bsum = bih + bhh
        bias = np.stack(
            [
                bsum[0 * H + c * 128:0 * H + (c + 1) * 128],
                bsum[1 * H + c * 128:1 * H + (c + 1) * 128],
                bih[2 * H + c * 128:2 * H + (c + 1) * 128],
                bhh[2 * H + c * 128:2 * H + (c + 1) * 128],
            ],
            axis=1,
        ).astype(np.float32)
        h0s = np.ascontiguousarray(seq[:, c * 128:(c + 1) * 128].T)
        gum_c = np.ascontiguousarray(gum[:, :, cs:cs + VS] + fcb[cs:cs + VS])
        in_maps.append(
            dict(
                fcT=fcT,
                wihT=np.ascontiguousarray(wih_t.reshape(128, 3 * KC_D * 128)),
                whhT=np.ascontiguousarray(whh_t.reshape(128, 3 * KC_H * 128)),
                bias=bias,
                h0T=h0T,
                h0s=h0s,
                x0T=x0T,
                gum=gum_c,
                emb=emb,
            )
        )
    return in_maps


def kernel(sequence_embedding, emb_table, W_ih, W_hh, b_ih, b_hh, fc_w, fc_b,
           eos_index):
    from concourse.bass_utils import run_bass_kernel_spmd

    gum = _gumbel_cpu()
    in_maps = _per_core_inputs(
        sequence_embedding, emb_table, W_ih, W_hh, b_ih, b_hh, fc_w, fc_b,
        eos_index, gum,
    )
    nc = _get_nc()
    res = run_bass_kernel_spmd(nc, in_maps, core_ids=list(range(NCORES)))
    fcb = np.asarray(fc_b, np.float32)
    logits = np.concatenate([r["logits_o"] for r in res.results], axis=2)
    logits += fcb  # exact no-op for fc_b == 0, matches reference otherwise
    indices = res.results[0]["idx_o"].astype(np.int32)
    return indices, logits
